# revision 16
# baseline (speedup 1.0000x reference)
"""Trainium2 Bass kernel for nn_NMPN (GNN message passing), 8 NeuronCores.

Algorithm (reference):
    h0 = relu(fatoms @ W_nin.T)                       [50000, 512]
    H = h0
    repeat 4x:
        msg_h = concat([zeros(1,512), H[in_n]])        [120000, 512]
        msg   = concat([msg_h, fbonds], 1)             [120000, 523]
        nei   = msg[aoutgraph].sum(1)                  [50000, 523]
        H     = relu(h0 + nei @ W_node.T)              [50000, 512]
    return H.T

Restructuring used here:
  - composite index src[a,j] = in_n[aoutgraph[a,j]-1] (or a zero row when
    aoutgraph==0), so each depth is a single gather-sum from an H table.
  - the fbonds part of nei is depth-invariant: base = h0 + (sum_j
    fbonds[aout[:,j]]) @ W_node[:,512:].T is computed once; per depth
    H = relu(base + nei_h @ W_h.T) with W_h = W_node[:,:512].
  - sharding: atoms row-sharded over 8 cores (6250 each). The H table
    (bf16, [50001, 512], row 50000 = zeros) is replicated per HBM pair in
    "Shared" scratchpad; an 8-rank AllGather refreshes it each depth
    (double-buffered A/B to avoid cross-core write-after-read races).
  - per 128-atom chunk: one indirect DMA gathers 6*128 rows; PE
    transpose-matmuls (bf16, vs identity) both transpose and 6-way
    accumulate them into PSUM (fp32); the main matmul runs in fp32r.
"""

import os
import numpy as np

import concourse.bass as bass
import concourse.mybir as mybir
import concourse.tile as tile
from concourse import bacc
from concourse.bass_utils import run_bass_kernel_spmd
from concourse.masks import make_identity

NCORES = 8
N_ATOMS = int(os.environ.get("TRN_N_ATOMS", "50000"))
N_BONDS = int(os.environ.get("TRN_N_BONDS", "120000"))
MAX_NB = 6
ATOM_FDIM = 39
BOND_FDIM = 11
HIDDEN = 512
DEPTH = int(os.environ.get("TRN_DEPTH", "4"))

A_LOC = N_ATOMS // NCORES            # 6250 atoms per core
NCHUNK = (A_LOC + 127) // 128        # 49 chunks
LAST_M = A_LOC - (NCHUNK - 1) * 128  # 106 atoms in last chunk
ZROW = N_ATOMS                       # index of the all-zero table row
FB = MAX_NB * BOND_FDIM              # 66

F32 = mybir.dt.float32
F32R = mybir.dt.float32r
BF16 = mybir.dt.bfloat16
I32 = mybir.dt.int32


def _chunk_m(c):
    return 128 if c < NCHUNK - 1 else LAST_M


def build_nc():
    nc = bacc.Bacc("TRN2", target_bir_lowering=False, num_devices=NCORES)

    # ---- per-core external I/O ----
    fatoms_t = nc.dram_tensor("fatoms_t", [ATOM_FDIM, A_LOC], F32R, kind="ExternalInput")
    fbg_t = nc.dram_tensor("fbg_t", [FB, A_LOC], F32R, kind="ExternalInput")
    src_idx = nc.dram_tensor("src_idx", [128, NCHUNK * MAX_NB], I32, kind="ExternalInput")
    w_nin_t = nc.dram_tensor("w_nin_t", [ATOM_FDIM, HIDDEN], F32R, kind="ExternalInput")
    wb_rep = nc.dram_tensor("wb_rep", [FB, HIDDEN], F32R, kind="ExternalInput")
    w_h_t = nc.dram_tensor("w_h_t", [HIDDEN, HIDDEN], F32R, kind="ExternalInput")
    h_out = nc.dram_tensor("h_out", [A_LOC, HIDDEN], F32, kind="ExternalOutput")

    # ---- internal DRAM ----
    debug_dump = bool(int(os.environ.get("TRN_DEBUG_DUMP", "0")))
    dbg_rows = dbg_g = None
    if debug_dump:
        dbg_rows = nc.dram_tensor("dbg_rows", [3, HIDDEN], BF16, kind="ExternalOutput")
        dbg_g = nc.dram_tensor("dbg_g", [NCHUNK, MAX_NB * HIDDEN], BF16, kind="ExternalOutput")
        dbg_gf = nc.dram_tensor("dbg_gf", [NCHUNK, 128, MAX_NB * HIDDEN], BF16, kind="ExternalOutput")
        dbg_nt = nc.dram_tensor("dbg_nt", [NCHUNK, HIDDEN], F32, kind="ExternalOutput")
        dbg_ntf = nc.dram_tensor("dbg_ntf", [NCHUNK, 128, HIDDEN], F32, kind="ExternalOutput")
        dbg_ntv = nc.dram_tensor("dbg_ntv", [NCHUNK, 128, HIDDEN], F32, kind="ExternalOutput")
        dbg_base = nc.dram_tensor("dbg_base", [NCHUNK, HIDDEN], F32, kind="ExternalOutput")
        dbg_tnew = nc.dram_tensor("dbg_tnew", [NCHUNK, HIDDEN], F32, kind="ExternalOutput")

    agin = nc.dram_tensor("agin", [A_LOC, HIDDEN], BF16)
    shared_tables = bool(int(os.environ.get("TRN_SHARED_TABLES", "0")))
    tables = [
        nc.dram_tensor(
            f"table{i}", [N_ATOMS + 1, HIDDEN], BF16,
            addr_space="Shared" if shared_tables else "Local",
        )
        for i in range(2)
    ]
    seed_dram = nc.dram_tensor("seed_dram", [1, 16], F32)

    rg = [list(range(NCORES))]

    with tile.TileContext(nc) as tc:
        with (
            tc.tile_pool(name="persist", bufs=1) as pp,
            tc.tile_pool(name="psum", bufs=2, space="PSUM") as psp,
            tc.tile_pool(name="work", bufs=3) as wp,
            tc.tile_pool(name="out", bufs=3) as op,
        ):
            # ---------- resident tiles ----------
            base_t = pp.tile([128, NCHUNK * HIDDEN], F32, tag="base")
            ident = pp.tile([128, 128], BF16, tag="ident")
            make_identity(nc, ident[:, :])
            src_sb = pp.tile([128, NCHUNK * MAX_NB], I32, tag="src")
            src_load = nc.sync.dma_start(out=src_sb[:, :], in_=src_idx[:, :])
            whs = pp.tile([128, 4 * HIDDEN], F32R, tag="wh")
            for b in range(4):
                nc.sync.dma_start(
                    out=whs[:, b * HIDDEN:(b + 1) * HIDDEN],
                    in_=w_h_t[b * 128:(b + 1) * 128, :],
                )
            zeros_bf = pp.tile([1, HIDDEN], BF16, tag="zr")
            nc.vector.memset(zeros_bf[:, :], 0.0)
            zeros_f = pp.tile([1, 16], F32, tag="zf")
            nc.vector.memset(zeros_f[:, :], 0.0)
            nc.sync.dma_start(out=seed_dram[:, :], in_=zeros_f[:, :])
            zrow_w = [
                nc.sync.dma_start(out=t[ZROW:ZROW + 1, :], in_=zeros_bf[:, :])
                for t in tables
            ]
            # base rows beyond A_LOC in the last chunk stay unread garbage
            # unless zeroed; memset keeps the sim finite-check happy.
            nc.vector.memset(base_t[:, :], 0.0)

            # ---------- setup: base = relu(fatoms@Wnin.T) + fbgather@Wbrep ----------
            with tc.tile_pool(name="setup", bufs=3) as sp:
                wnin_sb = pp.tile([ATOM_FDIM, HIDDEN], F32R, tag="wnin")
                nc.sync.dma_start(out=wnin_sb[:, :], in_=w_nin_t[:, :])
                wbr_sb = pp.tile([FB, HIDDEN], F32R, tag="wbr")
                nc.sync.dma_start(out=wbr_sb[:, :], in_=wb_rep[:, :])

                for c in range(NCHUNK):
                    m = _chunk_m(c)
                    a0 = c * 128
                    fa_sb = sp.tile([ATOM_FDIM, 128], F32R, tag="fa")
                    nc.sync.dma_start(out=fa_sb[:, :m], in_=fatoms_t[:, a0:a0 + m])
                    fbg_sb = sp.tile([FB, 128], F32R, tag="fbg")
                    nc.sync.dma_start(out=fbg_sb[:, :m], in_=fbg_t[:, a0:a0 + m])
                    ps_h0 = psp.tile([128, HIDDEN], F32, tag="ps_h0")
                    nc.tensor.matmul(
                        out=ps_h0[:m, :],
                        lhsT=fa_sb[:, :m],
                        rhs=wnin_sb[:, :],
                        start=True, stop=True,
                    )
                    ps_b = psp.tile([128, HIDDEN], F32, tag="ps_b")
                    nc.tensor.matmul(
                        out=ps_b[:m, :],
                        lhsT=fbg_sb[:, :m],
                        rhs=wbr_sb[:, :],
                        start=True, stop=True,
                    )
                    h0f = op.tile([128, HIDDEN], F32, tag="h0f")
                    nc.scalar.activation(
                        h0f[:m, :], ps_h0[:m, :],
                        mybir.ActivationFunctionType.Relu,
                    )
                    nc.vector.tensor_add(
                        base_t[:m, c * HIDDEN:(c + 1) * HIDDEN],
                        h0f[:m, :], ps_b[:m, :],
                    )
                    h0b = op.tile([128, HIDDEN], BF16, tag="h0b")
                    nc.vector.tensor_copy(h0b[:m, :], h0f[:m, :])
                    nc.sync.dma_start(out=agin[a0:a0 + m, :], in_=h0b[:m, :])

            # initial table: AllGather h0 (bf16) into table0
            cc = nc.gpsimd.collective_compute(
                "AllGather", mybir.AluOpType.bypass,
                replica_groups=rg, ins=[agin[:, :]], outs=[tables[0][0:N_ATOMS, :]],
            )

            # seed the Pool sequencer clock so the indirect gathers (1-wait
            # DMA instructions) need no extra waits of their own.
            def seed(dep_insts, tag):
                prev = None
                for i, d in enumerate(dep_insts):
                    st = wp.tile([1, 16], F32, tag=f"seed_{tag}_{i}")
                    s = nc.gpsimd.dma_start(out=st[:, :], in_=seed_dram[:, :])
                    tile.add_dep_helper(s.ins, d.ins, sync=True, reason=f"seed {tag}")
                    if prev is not None:
                        tile.add_dep_helper(s.ins, prev.ins, sync=False, reason="chain")
                    prev = s
                return prev

            seed([src_load] + zrow_w + [cc], "init")

            # ---------- depth loop ----------
            for d in range(DEPTH):
                t_in = tables[d % 2]
                last = d == DEPTH - 1
                if debug_dump and d == 0:
                    nc.sync.dma_start(out=dbg_rows[0:1, :], in_=t_in[0:1, :])
                    nc.sync.dma_start(out=dbg_rows[1:2, :], in_=t_in[1:2, :])
                    nc.sync.dma_start(out=dbg_rows[2:3, :], in_=t_in[ZROW:ZROW + 1, :])
                store_dmas = []
                for c in range(NCHUNK):
                    m = _chunk_m(c)
                    a0 = c * 128
                    g = wp.tile([128, MAX_NB * HIDDEN], BF16, tag="g")
                    if int(os.environ.get("TRN_GATHER_SPLIT", "0")):
                        for j in range(MAX_NB):
                            nc.gpsimd.indirect_dma_start(
                                out=g[:, j * HIDDEN:(j + 1) * HIDDEN],
                                out_offset=None,
                                in_=t_in[:, :],
                                in_offset=bass.IndirectOffsetOnAxis(
                                    ap=src_sb[:, c * MAX_NB + j:c * MAX_NB + j + 1], axis=0
                                ),
                            )
                    else:
                        nc.gpsimd.indirect_dma_start(
                            out=g[:, :],
                            out_offset=None,
                            in_=t_in[:, :],
                            in_offset=bass.IndirectOffsetOnAxis(
                                ap=src_sb[:, c * MAX_NB:(c + 1) * MAX_NB], axis=0
                            ),
                        )
                    if debug_dump and d == 0:
                        nc.sync.dma_start(out=dbg_g[c:c + 1, :], in_=g[0:1, :])
                        nc.sync.dma_start(out=dbg_gf[c, :, :], in_=g[:, :])
                    # transpose-accumulate the 6 gathered tiles: PSUM(neiT)
                    ps_nt = psp.tile([128, HIDDEN], F32, tag="ps_nt")
                    for b in range(4):
                        for j in range(MAX_NB):
                            nc.tensor.matmul(
                                out=ps_nt[:, b * 128:(b + 1) * 128],
                                lhsT=g[:, j * HIDDEN + b * 128: j * HIDDEN + (b + 1) * 128],
                                rhs=ident[:, :],
                                start=(j == 0), stop=(j == MAX_NB - 1),
                            )
                    nt = wp.tile([128, HIDDEN], F32R, tag="nt")
                    nc.scalar.copy(nt[:, :], ps_nt[:, :])
                    # H_new[a,:] = relu(base + neiT.T @ W_h.T)
                    ps_o = psp.tile([128, HIDDEN], F32, tag="ps_o")
                    for b in range(4):
                        nc.tensor.matmul(
                            out=ps_o[:, :],
                            lhsT=nt[:, b * 128:(b + 1) * 128],
                            rhs=whs[:, b * HIDDEN:(b + 1) * HIDDEN],
                            start=(b == 0), stop=(b == 3),
                        )
                    tnew = op.tile([128, HIDDEN], F32, tag="tnew")
                    nc.vector.tensor_add(
                        tnew[:, :], ps_o[:, :], base_t[:, c * HIDDEN:(c + 1) * HIDDEN]
                    )
                    if debug_dump and d == 0:
                        nc.gpsimd.dma_start(out=dbg_nt[c:c + 1, :], in_=nt[0:1, :])
                        nc.gpsimd.dma_start(out=dbg_ntf[c, :, :], in_=nt[:, :])
                        ntv = op.tile([128, HIDDEN], F32, tag="ntv")
                        nc.vector.tensor_copy(ntv[:, :], ps_nt[:, :])
                        nc.sync.dma_start(out=dbg_ntv[c, :, :], in_=ntv[:, :])
                        nc.sync.dma_start(out=dbg_base[c:c + 1, :], in_=base_t[0:1, c * HIDDEN:(c + 1) * HIDDEN])
                        nc.sync.dma_start(out=dbg_tnew[c:c + 1, :], in_=tnew[0:1, :])
                    if last:
                        hf = op.tile([128, HIDDEN], F32, tag="hf")
                        nc.scalar.activation(
                            hf[:, :], tnew[:, :], mybir.ActivationFunctionType.Relu
                        )
                        sd = nc.sync.dma_start(out=h_out[a0:a0 + m, :], in_=hf[:m, :])
                    else:
                        hb = op.tile([128, HIDDEN], BF16, tag="hb")
                        nc.scalar.activation(
                            hb[:, :], tnew[:, :], mybir.ActivationFunctionType.Relu
                        )
                        sd = nc.sync.dma_start(out=agin[a0:a0 + m, :], in_=hb[:m, :])
                    store_dmas.append(sd)

                if not last:
                    cc = nc.gpsimd.collective_compute(
                        "AllGather", mybir.AluOpType.bypass,
                        replica_groups=rg,
                        ins=[agin[:, :]],
                        outs=[tables[(d + 1) % 2][0:N_ATOMS, :]],
                    )
                    seed([cc], f"d{d}")

    nc.finalize()
    return nc


def _prepare_inputs(fatoms, fbonds, W_nin, W_node, aoutgraph, in_n):
    fatoms = np.asarray(fatoms, dtype=np.float32)
    fbonds = np.asarray(fbonds, dtype=np.float32)
    W_nin = np.asarray(W_nin, dtype=np.float32)
    W_node = np.asarray(W_node, dtype=np.float32)
    aout = np.asarray(aoutgraph, dtype=np.int64)
    in_n = np.asarray(in_n, dtype=np.int64)

    # composite source-atom index per (atom, neighbor); 0 -> zero row
    src = np.where(aout > 0, in_n[np.maximum(aout - 1, 0)], ZROW).astype(np.int32)

    w_nin_t = np.ascontiguousarray(W_nin.T)                       # [39, 512]
    w_h_t = np.ascontiguousarray(W_node[:, :HIDDEN].T)            # [512, 512]
    # wb_rep[j*11+f, o] = W_node[o, 512+f]
    wb = W_node[:, HIDDEN:]                                       # [512, 11]
    wb_rep = np.ascontiguousarray(np.tile(wb.T, (MAX_NB, 1)))     # [66, 512]

    in_maps = []
    for k in range(NCORES):
        sh = slice(k * A_LOC, (k + 1) * A_LOC)
        fat = np.ascontiguousarray(fatoms[sh].T)                  # [39, 6250]
        fbg = fbonds[aout[sh]].reshape(A_LOC, FB)                 # [6250, 66]
        fbg_t = np.ascontiguousarray(fbg.T)                       # [66, 6250]
        src_k = src[sh]                                           # [6250, 6]
        arr = np.full((128, NCHUNK * MAX_NB), ZROW, dtype=np.int32)
        for c in range(NCHUNK):
            m = _chunk_m(c)
            blk = src_k[c * 128:c * 128 + m]                      # [m, 6]
            arr[:m, c * MAX_NB:(c + 1) * MAX_NB] = blk
        in_maps.append({
            "fatoms_t": fat,
            "fbg_t": fbg_t,
            "src_idx": arr,
            "w_nin_t": w_nin_t,
            "wb_rep": wb_rep,
            "w_h_t": w_h_t,
        })
    return in_maps


_cached_nc = None


def _get_nc():
    global _cached_nc
    if _cached_nc is None:
        _cached_nc = build_nc()
    return _cached_nc


def run(inputs, trace=False):
    in_maps = _prepare_inputs(**inputs)
    nc = _get_nc()
    res = run_bass_kernel_spmd(
        nc, in_maps, core_ids=list(range(NCORES)), trace=trace
    )
    h_full = np.concatenate([res.results[c]["h_out"] for c in range(NCORES)], axis=0)
    out = np.ascontiguousarray(h_full.T)
    return out, res


def kernel(**inputs) -> np.ndarray:
    trace = bool(int(os.environ.get("TRN_KERNEL_TRACE", "0")))
    out, _ = run(inputs, trace=trace)
    return out
